# revision 18
# baseline (speedup 1.0000x reference)
"""Multi-head causal attention (B=4, T=1024, C=1024, H=16, D=64) on 8 TRN2 cores.

Sharding: tensor-parallel over heads. Core i owns heads {2i, 2i+1}:
  - x is replicated (sent pre-transposed as xT [C, B*T], bf16)
  - Wq/Wk/Wv sharded over heads -> per-core [C, 128] (2 heads concat on D)
  - row-parallel output projection: per-core Wp rows [128, C]; host sums the
    8 partial [B*T, C] outputs (the all-reduce) and adds bp.

Per-core kernel (bf16 matmuls, fp32 PSUM accumulation):
  for b in 0..3:
    qT/kT/vT [128(2 heads' d), 1024(t)] = W.T @ x[b].T    (PE; q/k halves
      accumulate k-outer over 3 live psums so b0's matmuls start as soon as
      the first x chunk lands; b0 x streams as 8 per-chunk DMAs interleaved
      across the sync+scalar HWDGE queues with the weight loads)
    V_aug [s, 2x(64 v + ones col)] via PE transpose + one strided DVE copy
    per head h:
      scoresT [s_chunk=128, t] = K Q^T   (skip fully-causal-masked tiles,
                                          shrunken t-ranges on diagonal)
      expT = exp(scores/32)              (ACT, fused scale; no max-subtract:
                                          |scores|<~1 so exp is safe)
      diagonal 128-blocks *= upper-tri mask (GpSimd)
      outT_aug [65, t] += V_aug[h].T @ expT    (accumulate over s chunks;
                                          row 64 = softmax denominator;
                                          4 po psum banks so this never
                                          waits on the previous normalize)
      denom -> packed reciprocal.  Hidden chains use DMA pack/unpack via
      DRAM; the final (exposed) chain runs entirely on PE: K=1 pack
      matmuls -> DVE reciprocal -> one unpack matmul -> one-hot selector
      matmuls to broadcast, so the tail never waits ~2.5us DMA hops.
      outT2[h*64:,:] = outT_aug[0:64] * rec2   (DVE)
    proj psum [t_tile 128, c 512] = outT2_tile.T @ Wp_l -> sbuf -> out DMA
    alternating between the sync and gpsimd queues.
"""

import ml_dtypes
import numpy as np

B, T, C = 4, 1024, 1024
H, D = 16, 64
NCORES = 8
HPC = H // NCORES      # heads per core = 2
D2 = HPC * D           # 128
BT = B * T
SCALE = 1.0 / np.sqrt(np.float32(C))  # 1/32
BF16 = ml_dtypes.bfloat16

_compiled = None


def _split_multi_waits(nc, mybir, maxw=1):
    """Walrus in this container encodes at most one sync wait per
    instruction (fp32 self-loading matmuls and drains overflow).  Hoist
    excess waits onto same-engine NoOps inserted just before."""
    for fn in nc.m.functions:
        for bb in fn.blocks:
            new = []
            for inst in bb.instructions:
                si = inst.sync_info
                waits = list(si.on_wait) if (si is not None and si.on_wait) else []
                if len(waits) > maxw:
                    extra, keep = waits[:-maxw], waits[-maxw:]
                    for j, w in enumerate(extra):
                        new.append(
                            mybir.InstNoOp(
                                name=f"{inst.name}-wsplit{j}",
                                engine=inst.engine,
                                sync_info=mybir.SyncInfo(on_wait=[w], on_update=[]),
                                bass_nofuse=True,
                            )
                        )
                    inst.sync_info = mybir.SyncInfo(
                        on_wait=keep,
                        on_update=list(si.on_update) if si.on_update else [],
                    )
                new.append(inst)
            bb.instructions = new


_LDW_OPT = False


def _patch_ldw_opt():
    """Let walrus dedup back-to-back LDWEIGHTS of the same stationary."""
    import concourse.bass_utils as _bu

    if getattr(_bu, "_ldw_opt_patched", False):
        return
    _orig = _bu.run_command

    def _run(argv, **kw):
        if _LDW_OPT and isinstance(argv, list):
            argv = [
                "--enable-ldw-opt=true" if x == "--enable-ldw-opt=false" else x
                for x in argv
            ]
        return _orig(argv, **kw)

    _bu.run_command = _run
    _bu._ldw_opt_patched = True


def _build():
    import concourse.bass as bass
    import concourse.mybir as mybir
    import concourse.tile as tile

    _patch_ldw_opt()

    f32 = mybir.dt.float32
    bf = mybir.dt.bfloat16
    EXP = mybir.ActivationFunctionType.Exp

    nc = bass.Bass("TRN2", target_bir_lowering=False, debug=False, num_devices=NCORES)

    KC = C // 128  # 8 contraction chunks over C
    NS = T // 128  # 8 s-chunks
    NH = 2         # two 512-wide t halves

    xT_d = nc.dram_tensor("xT", [C, BT], bf, kind="ExternalInput").ap()
    # weights are host-repacked to [128, KC*D2] so their DMA is 128x2KB
    # contiguous descriptors instead of 1024x256B (which clogs the queues
    # for ~5us at startup)
    wq_d = nc.dram_tensor("wq", [128, KC * D2], bf, kind="ExternalInput").ap()
    wk_d = nc.dram_tensor("wk", [128, KC * D2], bf, kind="ExternalInput").ap()
    wv_d = nc.dram_tensor("wv", [128, KC * D2], bf, kind="ExternalInput").ap()
    wp_d = nc.dram_tensor("wp", [D2, C], bf, kind="ExternalInput").ap()
    mi_d = nc.dram_tensor("mi", [128, 256], bf, kind="ExternalInput").ap()
    ones_d = nc.dram_tensor("ones", [128, 64], bf, kind="ExternalInput").ap()
    sel_d = nc.dram_tensor("sel", [8, 512], bf, kind="ExternalInput").ap()
    out_d = nc.dram_tensor("out", [BT, C], bf, kind="ExternalOutput").ap()

    import concourse.bass as _bass

    with tile.TileContext(nc) as tc:
        with (
            tc.tile_pool(name="const", bufs=1) as constp,
            tc.tile_pool(name="xin", bufs=3) as xinp,
            tc.tile_pool(name="qkv", bufs=3) as qkvp,
            tc.tile_pool(name="vaug", bufs=4) as vaugp,
            tc.tile_pool(name="exps", bufs=10) as expp,
            tc.tile_pool(name="smalls", bufs=4) as smallp,
            tc.tile_pool(name="outt", bufs=2) as outtp,
            tc.tile_pool(name="pout", bufs=4) as poutp,
            tc.tile_pool(name="dram", bufs=2, space="DRAM") as dramp,
            tc.tile_pool(name="ps512", bufs=3, space="PSUM") as ps512,
            tc.tile_pool(name="psatt", bufs=4, space="PSUM") as psatt,
            tc.tile_pool(name="psvt", bufs=1, space="PSUM") as psvt,
        ):
            # ---- constants ----
            wq_s = constp.tile([128, KC, D2], bf, tag="wq")
            wk_s = constp.tile([128, KC, D2], bf, tag="wk")
            wv_s = constp.tile([128, KC, D2], bf, tag="wv")
            wp_s = constp.tile([128, C], bf, tag="wp")
            mi_s = constp.tile([128, 256], bf, tag="mi")
            ones_s = constp.tile([128, 64], bf, tag="ones")
            sel_s = constp.tile([8, 512], bf, tag="sel")
            mask_s = mi_s[:, 0:128]
            ident = mi_s[:, 128:256]

            xtiles = {}

            def load_x(b, split=False):
                """Allocate 4 pair-tiles for x[b] and dispatch their DMAs.
                Pair-granular tiles keep the consumer matmuls' dependencies
                at 2-chunk granularity; called one batch ahead so the DMA
                pipe never drains between batches."""
                tiles = [
                    xinp.tile([128, 2, T], bf, tag=f"xb{i}", name=f"xb{b}_{i}")
                    for i in range(4)
                ]
                for i in range(4):
                    eng = nc.sync if (split and i % 2 == 0) else nc.scalar
                    eng.dma_start(
                        tiles[i][:],
                        xT_d[
                            i * 256:(i + 1) * 256, b * T:(b + 1) * T
                        ].rearrange("(k p) m -> p k m", p=128),
                    )
                    if b == 0 and i == 0:
                        nc.sync.dma_start(wv_s[:], wv_d)
                    if b == 0 and i == 1:
                        nc.scalar.dma_start(mi_s[:], mi_d)
                xtiles[b] = tiles

            def emit_qkv(b):
                if b == 0:
                    # b0 start is DMA-latency bound: interleave the weight
                    # loads with pair x loads across both HWDGE queues.
                    nc.sync.dma_start(wq_s[:], wq_d)
                    nc.scalar.dma_start(wk_s[:], wk_d)
                    load_x(0, split=True)
                    nc.sync.dma_start(wp_s[:], wp_d)
                    nc.sync.dma_start(sel_s[:], sel_d)
                    nc.scalar.dma_start(ones_s[:], ones_d)
                xb = xtiles.pop(b)
                if b < B - 1:
                    load_x(b + 1)
                qT = [
                    qkvp.tile([128, 512], bf, tag=f"qT{i}", name=f"qT{b}_{i}")
                    for i in range(NH)
                ]
                kT = [
                    qkvp.tile([128, 512], bf, tag=f"kT{i}", name=f"kT{b}_{i}")
                    for i in range(NH)
                ]
                vT = [
                    qkvp.tile([128, 512], bf, tag=f"vT{i}", name=f"vT{b}_{i}")
                    for i in range(NH)
                ]

                def xs(k, half):
                    return xb[k // 2][:, k % 2, half * 512:(half + 1) * 512]

                # k-outer over 3 live psums (q half0/half1, k half0); the
                # rest k-inner.  b0: ~0.65us of PE work per 0.73us x chunk.
                psq = [
                    ps512.tile([128, 512], f32, tag="ps512", name=f"psq{b}_{i}")
                    for i in range(NH)
                ]
                psk0 = ps512.tile([128, 512], f32, tag="ps512", name=f"psk0_{b}")
                for k in range(KC):
                    st, sp = (k == 0), (k == KC - 1)
                    nc.tensor.matmul(
                        psq[0][:], wq_s[:, k, :], xs(k, 0), start=st, stop=sp
                    )
                    nc.tensor.matmul(
                        psq[1][:], wq_s[:, k, :], xs(k, 1), start=st, stop=sp
                    )
                    nc.tensor.matmul(
                        psk0[:], wk_s[:, k, :], xs(k, 0), start=st, stop=sp
                    )
                for half in range(NH):
                    nc.vector.tensor_copy(qT[half][:], psq[half][:])
                nc.vector.tensor_copy(kT[0][:], psk0[:])
                # k half1 + v halves: k-inner
                for w_s, oT, half in (
                    (wk_s, kT, 1), (wv_s, vT, 0), (wv_s, vT, 1)
                ):
                    ps = ps512.tile([128, 512], f32, tag="ps512")
                    for k in range(KC):
                        nc.tensor.matmul(
                            ps[:], w_s[:, k, :], xs(k, half),
                            start=(k == 0), stop=(k == KC - 1),
                        )
                    nc.vector.tensor_copy(oT[half][:], ps[:])
                # V_aug: [s 128, 132] per s-chunk; h0 = cols 0:64 + ones col
                # 64, h1 = cols 66:130 + ones col 130.  The transposes are
                # emitted just-in-time inside the scores loop so the PE never
                # sits through the transpose->copy serialization.
                vaug = vaugp.tile(
                    [128, NS, 132], bf, tag="vaug", name=f"vaug{b}"
                )
                nc.sync.dma_start(
                    vaug[:, :, 0:132].rearrange("p s (g c) -> p s g c", g=2)[
                        :, :, :, 64:65
                    ],
                    ones_d[:, 0:2 * NS].rearrange("p (s g o) -> p s g o", g=2, o=1),
                )
                return qT, kT, vT, vaug

            def emit_normalize_half(b, half, po_h, outT2):
                """DMA-chain packed reciprocal (hidden by later PE work)."""
                t0 = half * 512
                den2 = smallp.tile(
                    [1, 2 * 512], f32, tag="den2", name=f"den2_{b}_{half}"
                )
                for h in range(HPC):
                    nc.vector.tensor_copy(
                        den2[0:1, h * 512:(h + 1) * 512], po_h[h][64:65, 0:512]
                    )
                packed = smallp.tile(
                    [128, 8], f32, tag="packed", name=f"packed_{b}_{half}"
                )
                nc.sync.dma_start(packed[:], den2[0:1, :])
                recp = smallp.tile(
                    [128, 8], f32, tag="recp", name=f"recp_{b}_{half}"
                )
                nc.vector.reciprocal(recp[:], packed[:])
                rrow = dramp.tile(
                    [1, 1024], f32, tag="rrow", name=f"rrow_{b}_{half}"
                )
                nc.sync.dma_start(
                    rrow[0, :].rearrange("(p f) -> p f", p=128), recp[:]
                )
                for h in range(HPC):
                    hp = slice(h * 64, (h + 1) * 64)
                    rec2 = smallp.tile(
                        [64, 512], f32, tag="rec2", name=f"rec2_{b}_{half}_{h}"
                    )
                    nc.sync.dma_start(
                        rec2[:],
                        _bass.AP(
                            rrow[:].tensor,
                            rrow[:].offset + 512 * h,
                            [[0, 64], [1, 512]],
                        ),
                    )
                    nc.vector.tensor_mul(
                        outT2[hp, t0:t0 + 512], po_h[h][0:64, 0:512], rec2[:]
                    )

            def emit_normalize_pe(b, half, po_h, outT2):
                """All-on-PE packed reciprocal for the exposed tail chain:
                K=1 pack matmuls -> DVE recip -> unpack matmul -> one-hot
                selector broadcast matmuls.  ~1.7us of PE, no DMA hops."""
                t0 = half * 512
                den2b = smallp.tile(
                    [1, 1024], bf, tag="den2b", name=f"den2b_{b}_{half}"
                )
                for h in range(HPC):
                    nc.vector.tensor_copy(
                        den2b[0:1, h * 512:(h + 1) * 512], po_h[h][64:65, 0:512]
                    )
                # pack: pvd[p, f] = den[f*128 + p]
                pvd = ps512.tile([128, 512], f32, tag="ps512", name=f"pvd_{b}")
                for f in range(8):
                    nc.tensor.matmul(
                        pvd[:, f:f + 1],
                        den2b[0:1, f * 128:(f + 1) * 128],
                        ones_s[0:1, 0:1],
                        start=(f == 0), stop=(f == 7),
                    )
                recp8 = smallp.tile(
                    [128, 8], bf, tag="recp8", name=f"recp8_{b}_{half}"
                )
                with nc.allow_low_precision(reason="bf16 recip feeds bf16 matmul"):
                    nc.vector.reciprocal(recp8[:], pvd[:, 0:8])
                # unpack: rr8[f, t'] = recp8[t', f] via recp8^T @ I
                rr8ps = ps512.tile([128, 512], f32, tag="ps512", name=f"rr8ps_{b}")
                nc.tensor.matmul(
                    rr8ps[0:8, 0:128], recp8[:], ident[:], start=True, stop=True
                )
                rr8 = smallp.tile([8, 128], bf, tag="rr8", name=f"rr8_{b}_{half}")
                nc.vector.tensor_copy(rr8[:], rr8ps[0:8, 0:128])
                for h in range(HPC):
                    hp = slice(h * 64, (h + 1) * 64)
                    psb = ps512.tile(
                        [128, 512], f32, tag="ps512", name=f"psb_{b}_{h}"
                    )
                    for c in range(4):
                        f = h * 4 + c
                        nc.tensor.matmul(
                            psb[0:64, c * 128:(c + 1) * 128],
                            sel_s[:, f * 64:(f + 1) * 64],
                            rr8[:],
                            start=(c == 0), stop=(c == 3),
                        )
                    rec2 = smallp.tile(
                        [64, 512], bf, tag="rec2b", name=f"rec2b_{b}_{half}_{h}"
                    )
                    nc.scalar.copy(rec2[:], psb[0:64, :])
                    nc.vector.tensor_mul(
                        outT2[hp, t0:t0 + 512], po_h[h][0:64, 0:512], rec2[:]
                    )

            def emit_attnv0(b, po0, vaug, exs, sl):
                for h in range(HPC):
                    nc.tensor.matmul(
                        po0[h][0:65, sl * 128:512],
                        vaug[:, sl, 66 * h:66 * h + 65],
                        exs[(h, sl)][:, 0:512 - sl * 128],
                        start=(sl == 0),
                        stop=(sl == 3),
                    )

            def emit_scores(b, qT, kT, vT, vaug, outT2):
                """Full-width scores/exp per (h, s): one [128, 1024-s0] bf16
                matmul + exp.  The V transpose for chunk s and the half0
                attnV accumulation for chunk s-1 are interleaved into the
                stream so the PE always has runnable work while DVE/ACT
                chase the exp/mask/copy chain."""
                exs = {}
                po0 = [
                    psatt.tile([128, 512], f32, tag="psatt", name=f"po0_{b}_{h}")
                    for h in range(HPC)
                ]
                for s in range(NS):
                    s0 = s * 128
                    d1 = max(0, s0 - 512)
                    # just-in-time V transpose: copy lands while the scores
                    # matmuls below stream
                    pv = psvt.tile([128, 128], bf, tag="psvt")
                    so = (s % 4) * 128
                    nc.tensor.transpose(
                        pv[:], vT[s // 4][:, so:so + 128], ident
                    )
                    dst = vaug[:, s, 0:132].rearrange("p (g c) -> p g c", g=2)[
                        :, :, 0:64
                    ]
                    vsrc = pv[:].rearrange("p (g c) -> p g c", g=2)
                    if s % 2 == 0:
                        nc.vector.tensor_copy(dst, vsrc)
                    else:
                        nc.scalar.copy(dst, vsrc)
                    for h in range(HPC):
                        hp = slice(h * 64, (h + 1) * 64)
                        ex = expp.tile(
                            [128, 1024], bf, tag="ex", bufs=18,
                            name=f"ex{b}_{h}_{s}"
                        )
                        exs[(h, s)] = ex
                        kslc = kT[s // 4][hp, (s0 % 512):(s0 % 512) + 128]
                        if s < 4:  # t-half0 piece: cols [s0, 512)
                            w0 = 512 - s0
                            pa = ps512.tile([128, 512], f32, tag="ps512")
                            nc.tensor.matmul(
                                pa[:, 0:w0],
                                kslc,
                                qT[0][hp, s0:512],
                                start=True,
                                stop=True,
                            )
                            nc.scalar.activation(
                                ex[:, 0:w0], pa[:, 0:w0], EXP, scale=float(SCALE)
                            )
                        # t-half1 piece: cols [max(512, s0), 1024)
                        w1 = 512 - d1
                        pb = ps512.tile([128, 512], f32, tag="ps512")
                        nc.tensor.matmul(
                            pb[:, 0:w1],
                            kslc,
                            qT[1][hp, d1:512],
                            start=True,
                            stop=True,
                        )
                        nc.scalar.activation(
                            ex[:, 512 - s0 + d1:T - s0],
                            pb[:, 0:w1],
                            EXP,
                            scale=float(SCALE),
                        )
                        nc.gpsimd.tensor_mul(
                            ex[:, 0:128], ex[:, 0:128], mask_s[:]
                        )
                    if 1 <= s <= 4:
                        emit_attnv0(b, po0, vaug, exs, s - 1)
                    if s == 4:
                        # attnV half0 is complete; start its normalize chain
                        # now so it overlaps scores s=5..7.  The last batch
                        # uses the all-PE chain (its mid-proj would otherwise
                        # wait out the DMA hops).
                        if b == B - 1:
                            emit_normalize_pe(b, 0, po0, outT2)
                        else:
                            emit_normalize_half(b, 0, po0, outT2)
                return exs, po0

            def emit_attnv_half1(b, vaug, exs, mid=None, mid_s=3):
                po1 = [
                    psatt.tile([128, 512], f32, tag="psatt", name=f"po1_{b}_{h}")
                    for h in range(HPC)
                ]
                for s in range(NS):
                    s0 = s * 128
                    d1 = max(0, s0 - 512)
                    for h in range(HPC):
                        nc.tensor.matmul(
                            po1[h][0:65, d1:512],
                            vaug[:, s, 66 * h:66 * h + 65],
                            exs[(h, s)][:, 512 - s0 + d1:T - s0],
                            start=(s == 0),
                            stop=(s == NS - 1),
                        )
                    if s == mid_s and mid is not None:
                        mid()  # e.g. last b's proj half0 fills the PE stream
                return po1

            def emit_proj_half(b, outT2, half, final=False):
                # row-parallel projection: [t 128, c 512] tiles for this half's t
                for tt in range(half * 4, half * 4 + 4):
                    ob = poutp.tile([128, C], bf, tag="ob")
                    for ct in range(2):
                        pp = ps512.tile([128, 512], f32, tag="ps512")
                        nc.tensor.matmul(
                            pp[:],
                            outT2[:, tt * 128:(tt + 1) * 128],
                            wp_s[:, ct * 512:(ct + 1) * 512],
                            start=True,
                            stop=True,
                        )
                        if ct == 0:
                            nc.scalar.copy(ob[:, ct * 512:(ct + 1) * 512], pp[:])
                        else:
                            nc.vector.tensor_copy(
                                ob[:, ct * 512:(ct + 1) * 512], pp[:]
                            )
                    # the final half fans its stores across both spare queues
                    # so the drain tail is not serialized on one DGE
                    eng = nc.sync if (final and tt % 2 == 0) else nc.gpsimd
                    eng.dma_start(
                        out_d[b * T + tt * 128:b * T + (tt + 1) * 128, :], ob[:]
                    )

            # Software pipeline: proj halves of b-1 are emitted between b's
            # stages so the in-order PE stream never waits on the normalize
            # chain (DVE/DMA) of the batch it just finished.
            prev = None
            for b in range(B):
                qT, kT, vT, vaug = emit_qkv(b)
                outT2 = outtp.tile([128, T], bf, tag="outT2", name=f"outT2_{b}")
                if prev is not None:
                    emit_proj_half(prev[0], prev[1], 0)
                exs, po0 = emit_scores(b, qT, kT, vT, vaug, outT2)
                if prev is not None:
                    emit_proj_half(prev[0], prev[1], 1)
                last = b == B - 1
                po1 = emit_attnv_half1(
                    b, vaug, exs,
                    mid=(lambda: emit_proj_half(b, outT2, 0)) if last else None,
                    mid_s=5,
                )
                if last:
                    emit_normalize_pe(b, 1, po1, outT2)
                else:
                    emit_normalize_half(b, 1, po1, outT2)
                prev = (b, outT2)
            emit_proj_half(prev[0], prev[1], 1, final=True)

    _split_multi_waits(nc, mybir)
    return nc


def _get_compiled():
    global _compiled
    if _compiled is None:
        _compiled = _build()
    return _compiled


def _make_in_maps(x, Wq, Wk, Wv, Wp):
    xT = np.ascontiguousarray(
        np.asarray(x, dtype=np.float32).reshape(BT, C).T
    ).astype(BF16)  # [C, BT]
    mask = np.triu(np.ones((128, 128), dtype=BF16))  # keep j >= i
    ident = np.eye(128, dtype=BF16)
    mi = np.ascontiguousarray(np.concatenate([mask, ident], axis=1))
    ones = np.ones((128, 64), dtype=BF16)
    sel = np.zeros((8, 512), dtype=BF16)
    for f in range(8):
        sel[f, f * 64:(f + 1) * 64] = 1
    in_maps = []
    for i in range(NCORES):
        h0 = i * HPC
        def pack_w(W):
            # [H, C, D] heads -> [C, D2] -> DMA-friendly [128, KC*D2]:
            # partition p holds chunks k of row k*128+p
            w = (
                np.asarray(W[h0:h0 + HPC], dtype=np.float32)
                .transpose(1, 0, 2)
                .reshape(C, D2)
                .reshape(C // 128, 128, D2)
                .transpose(1, 0, 2)
                .reshape(128, (C // 128) * D2)
            )
            return np.ascontiguousarray(w).astype(BF16)

        wq = pack_w(Wq)
        wk = pack_w(Wk)
        wv = pack_w(Wv)
        wp = np.ascontiguousarray(
            np.asarray(Wp, dtype=np.float32)[h0 * D:(h0 + HPC) * D, :]
        ).astype(BF16)
        in_maps.append(
            {"xT": xT, "wq": wq, "wk": wk, "wv": wv, "wp": wp, "mi": mi,
             "ones": ones, "sel": sel}
        )
    return in_maps


def run(x, Wq, Wk, Wv, Wp, bp, trace=False, trace_cores=None):
    """Returns (full_output [B,T,C], BassKernelResults)."""
    from concourse.bass_utils import run_bass_kernel_spmd

    nc = _get_compiled()
    in_maps = _make_in_maps(x, Wq, Wk, Wv, Wp)
    kw = {}
    if trace:
        kw = {"trace": True, "trace_cores": trace_cores or [0]}
    res = run_bass_kernel_spmd(nc, in_maps, list(range(NCORES)), **kw)
    acc = np.zeros((BT, C), dtype=np.float32)
    for i in range(NCORES):
        acc += np.asarray(res.results[i]["out"], dtype=np.float32)
    acc += np.asarray(bp, dtype=np.float32)[None, :]
    return acc.reshape(B, T, C), res


def kernel(x, Wq, Wk, Wv, Wp, bp):
    out, _ = run(x, Wq, Wk, Wv, Wp, bp)
    return out
